# revision 26
# baseline (speedup 1.0000x reference)
"""Trainium2 Bass kernel: fractional Brownian motion kernel layer.

K[i,j] = 0.5 * sum_d (|x_id|^p + |X2_jd|^p - |x_id - X2_jd|^p),
p = 2*softplus(log_H),  x:[2048,16], X2:[2048,16] -> K:[2048,2048] f32.

Strategy: cosine-feature factorization onto the tensor engine. The 1-D
function f(t) = |t|^p is approximated on [-A, A] (A = max|x| + max|X2|,
fit solved on host per call) by

    f(t) ~= a0 + sum_k a_k cos(w_k t),   k = 1..12,  w_k = pi k / (1.2 A)

and cos(w(x-y)) = cos(wx)cos(wy) + sin(wx)sin(wy) is exactly separable, so

    K[i,j] = [0.5 t1_i - 8 a0]*1_j + 1_i*[0.5 t2_j] - sum_{d,k} (a_k/2)
             (cos(w_k x_id)cos(w_k X2_jd) + sin(w_k x_id)sin(w_k X2_jd))

which is ONE matmul with contraction 2*12*16 = 384 (3 tiles of 128) plus a
rank-2 "extras" matmul carrying the t1/t2 rows -- themselves produced by
tiny PE matmuls against the same feature tiles. PSUM accumulates all four
contraction passes; each [128,512] bank IS the final output tile.

Trig features: the ACT Sin spline is only valid on [-pi, pi] (HW-probed), so
arguments are computed in turn units u = w'_k v + phase (phase 0.25 on cos
rows -- per-partition scale/bias columns), range-reduced with the fp32
magic-constant round trick r = u - ((u + 1.5*2^23) - 1.5*2^23) in
[-0.5, 0.5], then Sin(2*pi*r). The tensor_tensor subtract runs on GpSimd
(otherwise idle) when GPS_TT is set, relieving the DVE.

Matmuls run in float32r (FP22) -- full bf16-rate at moving-dim >= 256,
mantissa 13 bits; end-to-end sim incl. f22 truncation: rel err 6.9e-3
vs the 2e-2 gate.

Sharding: rows of x across 8 cores (256 each), X2 replicated. Host-side
prep is layout-only (transpose + 8x partition replication) plus the tiny
12-coefficient fit (data-independent sizes).
"""

from contextlib import ExitStack

import numpy as np

import concourse.bass as bass
import concourse.tile as tile
from concourse import mybir, bacc
from concourse.bass_utils import run_bass_kernel_spmd

AF = mybir.ActivationFunctionType
OP = mybir.AluOpType
F32 = mybir.dt.float32
F32R = mybir.dt.float32r

N, M, D = 2048, 2048, 16
NCORES = 8
NS = N // NCORES          # 256 rows of x per core
P = 128                   # SBUF partitions
NIT = NS // P             # 2 i-blocks per core
JW = 512                  # PSUM bank width (fp32)
NJC = M // JW             # 4 j-chunks
KF = 12                   # cosine frequencies
CT = (2 * KF * D) // P    # 3 contraction tiles of 128
L_FAC = 1.2               # fit period factor: w_k = pi k / (L_FAC * A)
MAGIC = 12582912.0        # 1.5 * 2^23: fp32 round-to-nearest-int trick
TWO_PI = float(2.0 * np.pi)
MINUS_PI = float(-np.pi)
ZOFF = 8.0                # keeps u = w*v + ph'' nonnegative before mod

# layout of per-core constants: colsf [128, CFW] f32 (engine scalars),
# colsr [128, CRW] f32r (matmul lhsT vectors, host-truncated to fp22)
CW = 0        # 0..2   w'_k (turns per unit) per c-tile
CPH = 3       # 3..5   phase (0.25 cos rows / 0 sin rows)
CA = 6        # 6..8   -(a_k/2)  (phi feature scale)
CA0 = 9       # 8*a0 replicated
CMPI = 10     # -pi replicated (Sin bias AP)
CFW = 11
CV1 = 0       # 0..2   t1 lhsT vector (-1 on cos rows)
CV2 = 3       # 3..5   t2 lhsT vector (a_k/2 on cos rows)
CRW = 6

_CACHE = {}
# ablation switches for perf experiments: subsets of
# {"feat", "mms", "extras", "epilogue"} are DISABLED when present
ABLATE = set()
# run the range-reduction tensor_tensor subtract on GpSimd instead of DVE
# (measured ~16x slower there -- gpsimd serializes its 16 partitions/core)
GPS_TT = False


from contextlib import contextmanager


@contextmanager
def _pinned_act_tables():
    """Pin Sin/Copy to the single `trig_and_small` set so the act-table-load
    pass emits exactly one load. Scoped: restored right after compile."""
    import concourse.hw_specs as hw_specs
    import concourse.bacc as bacc_mod

    orig = bacc_mod.get_activation_tables
    base = hw_specs.get_activation_tables
    ours = {AF.Sin, AF.Identity, AF.Copy, AF.Abs, AF.Square}

    def patched(module_arch):
        tabs = {k: set(v) for k, v in base(module_arch).items()}
        for name, fns in tabs.items():
            if name != "trig_and_small":
                fns -= ours
        return tabs

    bacc_mod.get_activation_tables = patched
    try:
        yield
    finally:
        bacc_mod.get_activation_tables = orig


def _build_nc(reps=1, body_reps=1):
    with _pinned_act_tables():
        return _build_nc_inner(reps, body_reps)


def _build_nc_inner(reps=1, body_reps=1):
    nc = bacc.Bacc(trn_type="TRN2", target_bir_lowering=False, debug=False,
                   num_devices=NCORES)

    yrep = nc.declare_dram_parameter("yrep", [P, M], F32, isOutput=False)
    xrep = nc.declare_dram_parameter("xrep", [P, NS], F32, isOutput=False)
    colsf = nc.declare_dram_parameter("colsf", [P, CFW], F32, isOutput=False)
    colsr = nc.declare_dram_parameter("colsr", [P, CRW], F32R, isOutput=False)
    ones = nc.declare_dram_parameter("ones", [2, M], F32R, isOutput=False)
    out = nc.declare_dram_parameter("out", [NS, M], F32, isOutput=True)

    aps = tuple(h.ap() for h in (yrep, xrep, colsf, colsr, ones, out))

    with tile.TileContext(nc) as tc, ExitStack() as ctx:
        pools = {
            "const": ctx.enter_context(tc.tile_pool(name="const", bufs=2)),
            "xs": ctx.enter_context(tc.tile_pool(name="xs", bufs=3)),
            "ys": ctx.enter_context(tc.tile_pool(name="ys", bufs=3)),
            "phi": ctx.enter_context(tc.tile_pool(name="phi", bufs=2)),
            "psi": ctx.enter_context(tc.tile_pool(name="psi", bufs=2)),
            "stage": ctx.enter_context(tc.tile_pool(name="stage", bufs=2)),
            "ps": ctx.enter_context(tc.tile_pool(name="ps", bufs=1,
                                                 space="PSUM")),
        }

        if reps > 1:  # benchmark mode: repeat the whole body on-device
            ctx.enter_context(tc.For_i(0, reps, 1, staggered_reset=True))

        for _ in range(body_reps):
            _emit_body(nc, tc, pools, aps)

    nc.compile()
    return nc


def _emit_body(nc, tc, pools, aps):
    yrep_ap, xrep_ap, colsf_ap, colsr_ap, ones2_ap, out_ap = aps
    ones_ap = ones2_ap[0:1, :]
    a0row_ap = ones2_ap[1:2, :]
    const = pools["const"]

    # ---- inputs (yrep first: biggest and heads the sync-ring FIFO) ----
    yrepb = const.tile([P, M], F32, tag="yrep")
    nc.sync.dma_start(out=yrepb, in_=yrep_ap)
    colsb = const.tile([P, CFW], F32, tag="colsf")
    nc.sync.dma_start(out=colsb, in_=colsf_ap)
    colsr = const.tile([P, CRW], F32R, tag="colsr")
    nc.sync.dma_start(out=colsr, in_=colsr_ap)
    xrepb = const.tile([P, NS], F32, tag="xrep")
    nc.sync.dma_start(out=xrepb, in_=xrep_ap)

    # extras rows (engine partition bases must be 32-aligned, so rows >= 1
    # are filled by DMA): lxe = [ones, t1row, ones], rye = [raw t2 row
    # (= 0.5 t2 - 8 a0), ones, 8*a0] -- row2 pairs restore the a0 constant
    lxe = const.tile([3, NS], F32R, tag="lxe")
    rye = const.tile([3, M], F32R, tag="rye")
    nc.sync.dma_start(out=lxe[0:1, :], in_=ones_ap[0:1, 0:NS])
    nc.sync.dma_start(out=lxe[2:3, :], in_=ones_ap[0:1, 0:NS])
    nc.sync.dma_start(out=rye[1:2, :], in_=ones_ap)
    nc.sync.dma_start(out=rye[2:3, :], in_=a0row_ap)

    # ---- features: per c-tile t, partition p encodes (k, trig, d) ----
    phis, psis = [], []
    for t in range(CT):
        if "feat" in ABLATE:
            phi = pools["phi"].tile([P, NS], F32R, tag=f"phi{t}")
            nc.gpsimd.memset(phi, 0.25)
            phis.append(phi)
            psi = pools["psi"].tile([P, M], F32R, tag=f"psi{t}")
            nc.gpsimd.memset(psi, 0.25)
            psis.append(psi)
            continue
        wc = colsb[:, CW + t:CW + t + 1]
        phc = colsb[:, CPH + t:CPH + t + 1]
        ac = colsb[:, CA + t:CA + t + 1]

        # x side (small): same chain + coefficient scale, all on DVE
        ux = pools["xs"].tile([P, NS], F32, tag="ux")
        rx = pools["xs"].tile([P, NS], F32, tag="rx")
        nc.vector.tensor_scalar(out=ux, in0=xrepb, scalar1=wc, scalar2=phc,
                                op0=OP.mult, op1=OP.add)
        nc.vector.tensor_scalar(out=rx, in0=ux, scalar1=MAGIC, scalar2=MAGIC,
                                op0=OP.add, op1=OP.subtract)
        nc.vector.tensor_tensor(out=ux, in0=ux, in1=rx, op=OP.subtract)
        phi = pools["phi"].tile([P, NS], F32R, tag=f"phi{t}")
        nc.scalar.activation(out=phi, in_=ux, func=AF.Sin, scale=TWO_PI)
        nc.vector.tensor_scalar(out=phi, in0=phi, scalar1=ac, scalar2=None,
                                op0=OP.mult)
        phis.append(phi)

        # y side: u = w*y + ph (DVE 2-op), rnd = (u+M)-M (DVE 2-op),
        # r = u - rnd (GpSimd or DVE), Sin on ACT
        uy = pools["ys"].tile([P, M], F32, tag="uy")
        ry = pools["ys"].tile([P, M], F32, tag="ry")
        nc.scalar.activation(out=uy, in_=yrepb, func=AF.Identity,
                             bias=phc, scale=wc)
        nc.vector.tensor_scalar(out=ry, in0=uy, scalar1=MAGIC, scalar2=MAGIC,
                                op0=OP.add, op1=OP.subtract)
        tt_eng = nc.gpsimd if GPS_TT else nc.vector
        tt_eng.tensor_tensor(out=uy, in0=uy, in1=ry, op=OP.subtract)
        psi = pools["psi"].tile([P, M], F32R, tag=f"psi{t}")
        nc.scalar.activation(out=psi, in_=uy, func=AF.Sin, scale=TWO_PI)
        psis.append(psi)

    if "extras" in ABLATE:
        nc.gpsimd.memset(lxe, 0.5)
        nc.gpsimd.memset(rye, 0.5)
        _skip_extras = True
    else:
        _skip_extras = False
    # ---- t1 row: p1[0, i] = 0.5*t1_i - 8*a0; staged at partition 0,
    # then a tiny SBUF->SBUF DMA drops it into lxe partition 1 ----
    if not _skip_extras:
        p1 = pools["ps"].tile([1, NS], F32, tag="p1")
        for t in range(CT):
            nc.tensor.matmul(p1, colsr[:, CV1 + t:CV1 + t + 1], phis[t],
                             start=(t == 0), stop=(t == CT - 1))
        t1row = const.tile([1, NS], F32R, tag="t1row")
        nc.vector.tensor_copy(t1row, p1)
        nc.scalar.dma_start(out=lxe[1:2, :], in_=t1row)

    # ---- t2 row: rye[0, j] = 0.5*t2_j, two rotating [1, JW] banks ----
    if not _skip_extras:
        for jc in range(NJC):
            p2 = pools["ps"].tile([1, JW], F32, name=f"p2{jc}",
                                  tag=f"p2{jc % 2}")
            for t in range(CT):
                nc.tensor.matmul(p2, colsr[:, CV2 + t:CV2 + t + 1],
                                 psis[t][:, jc * JW:(jc + 1) * JW],
                                 start=(t == 0), stop=(t == CT - 1))
            nc.vector.tensor_copy(rye[0:1, jc * JW:(jc + 1) * JW], p2)

    # ---- main matmuls: bank (it, jc) accumulates 3 trig tiles + extras.
    # 6 rotating PSUM bank tags (+p1 +p2 = 8 total); epilogue per i-block
    # so reused banks are drained before the second block's matmuls. ----
    for it in range(NIT):
        banks = {}
        for c in range(CT + 1):
            if "mms" in ABLATE and c > 0:
                continue
            for jc in range(NJC):
                if jc not in banks:
                    n = it * NJC + jc
                    banks[jc] = pools["ps"].tile([P, JW], F32,
                                                 name=f"b{n % 5}",
                                                 tag=f"b{n % 5}")
                b = banks[jc]
                if "mms" in ABLATE:
                    nc.tensor.matmul(
                        b, lxe[:, it * P:(it + 1) * P],
                        rye[:, jc * JW:(jc + 1) * JW],
                        start=True, stop=True)
                elif c < CT:
                    nc.tensor.matmul(
                        b, phis[c][:, it * P:(it + 1) * P],
                        psis[c][:, jc * JW:(jc + 1) * JW],
                        start=(c == 0),
                        stop=(_skip_extras and c == CT - 1))
                elif not _skip_extras:
                    nc.tensor.matmul(
                        b, lxe[:, it * P:(it + 1) * P],
                        rye[:, jc * JW:(jc + 1) * JW],
                        start=False, stop=True)

        # epilogue: PSUM -> SBUF -> DRAM, split copies DVE/ACT
        for jc in range(NJC):
            st = pools["stage"].tile([P, JW], F32, name=f"st{it}_{jc}",
                                     tag=f"st{it}_{jc}")
            if jc % 2 == 0 or (it == 1 and jc == 1):
                nc.scalar.copy(st, banks[jc])
            else:
                nc.vector.tensor_copy(st, banks[jc])
            nc.scalar.dma_start(
                out=out_ap[it * P:(it + 1) * P, jc * JW:(jc + 1) * JW],
                in_=st)


def _get_nc(reps=1, body_reps=1):
    key = ("nc", reps, body_reps, tuple(sorted(ABLATE)), GPS_TT)
    if key not in _CACHE:
        _CACHE[key] = _build_nc(reps, body_reps)
    return _CACHE[key]


def _fit(p, A):
    """Minimax-ish (Lawson IRLS) cosine fit of t^p on [0, A]."""
    key = ("fit", round(float(p), 8), round(float(A), 5))
    if key in _CACHE:
        return _CACHE[key]
    t = np.linspace(0.0, A, 4001)
    f = t ** p
    w = np.pi * np.arange(1, KF + 1) / (L_FAC * A)
    Mx = np.concatenate([np.ones((len(t), 1)), np.cos(np.outer(t, w))], axis=1)
    wt = np.ones(len(t))
    coef = None
    for _ in range(60):
        coef, *_ = np.linalg.lstsq(Mx * wt[:, None], f * wt, rcond=None)
        r = Mx @ coef - f
        wt *= np.abs(r) + 1e-12
        wt /= wt.sum()
        wt = np.maximum(wt, 1e-14)
    _CACHE[key] = (float(coef[0]), coef[1:].copy(), w)
    return _CACHE[key]


def _make_in_maps(x, X2, log_H):
    x = np.ascontiguousarray(np.asarray(x, dtype=np.float32))
    X2 = np.ascontiguousarray(np.asarray(X2, dtype=np.float32))
    lh = float(np.asarray(log_H, dtype=np.float32))
    p = 2.0 * float(np.log1p(np.exp(lh)))
    A = float(np.abs(x).max() + np.abs(X2).max())
    a0, a, w = _fit(p, A)
    wt = (w / (2.0 * np.pi)).astype(np.float64)   # turns per unit

    cc = np.arange(2 * KF * D)
    k_of = cc // 32
    trig = (cc % 32) // 16                        # 0 = cos, 1 = sin
    colsf = np.zeros((P, CFW), np.float32)
    colsr = np.zeros((P, CRW), np.float32)
    for t in range(CT):
        s = slice(t * P, (t + 1) * P)
        colsf[:, CW + t] = wt[k_of[s]]
        colsf[:, CPH + t] = np.where(trig[s] == 0, 0.25, 0.0)
        colsf[:, CA + t] = -(a[k_of[s]] / 2.0)
        colsr[:, CV1 + t] = np.where(trig[s] == 0, -1.0, 0.0)
        colsr[:, CV2 + t] = np.where(trig[s] == 0, a[k_of[s]] / 2.0, 0.0)
    colsf[:, CA0] = 8.0 * a0
    colsf[:, CMPI] = MINUS_PI
    # truncate matmul weight vectors to fp22 so the bits are valid f32r
    colsr.view(np.uint32)[...] &= np.uint32(0xFFFFFC00)

    yrep = np.ascontiguousarray(np.tile(X2.T, (P // D, 1)))   # [128, 2048]
    in_maps = []
    for c in range(NCORES):
        xs = x[c * NS:(c + 1) * NS]
        xrep = np.ascontiguousarray(np.tile(xs.T, (P // D, 1)))  # [128, 256]
        ones2 = np.ones((2, M), np.float32)
        ones2[1, :] = 8.0 * a0
        ones2.view(np.uint32)[...] &= np.uint32(0xFFFFFC00)
        in_maps.append({"yrep": yrep, "xrep": xrep, "colsf": colsf,
                        "colsr": colsr, "ones": ones2})
    return in_maps


def run_spmd(x, X2, log_H, trace=False, reps=1, body_reps=1, **kw):
    nc = _get_nc(reps, body_reps)
    in_maps = _make_in_maps(x, X2, log_H)
    return run_bass_kernel_spmd(nc, in_maps, list(range(NCORES)),
                                trace=trace, **kw)


def kernel(x, X2, log_H):
    res = run_spmd(x, X2, log_H)
    return np.concatenate([res.results[c]["out"] for c in range(NCORES)],
                          axis=0)



# revision 27
# speedup vs baseline: 1.1612x; 1.1612x over previous
"""Trainium2 Bass kernel: fractional Brownian motion kernel layer.

K[i,j] = 0.5 * sum_d (|x_id|^p + |X2_jd|^p - |x_id - X2_jd|^p),
p = 2*softplus(log_H),  x:[2048,16], X2:[2048,16] -> K:[2048,2048] f32.

Strategy: cosine-feature factorization onto the tensor engine. The 1-D
function f(t) = |t|^p is approximated on [-A, A] (A = max|x| + max|X2|,
fit solved on host per call) by

    f(t) ~= a0 + sum_k a_k cos(w_k t),   k = 1..12,  w_k = pi k / (1.2 A)

and cos(w(x-y)) = cos(wx)cos(wy) + sin(wx)sin(wy) is exactly separable, so

    K[i,j] = [0.5 t1_i - 8 a0]*1_j + 1_i*[0.5 t2_j] - sum_{d,k} (a_k/2)
             (cos(w_k x_id)cos(w_k X2_jd) + sin(w_k x_id)sin(w_k X2_jd))

which is ONE matmul with contraction 2*12*16 = 384 (3 tiles of 128) plus a
rank-2 "extras" matmul carrying the t1/t2 rows -- themselves produced by
tiny PE matmuls against the same feature tiles. PSUM accumulates all four
contraction passes; each [128,512] bank IS the final output tile.

Trig features: the ACT Sin spline is only valid on [-pi, pi] (HW-probed), so
arguments are computed in turn units u = w'_k v + phase (phase 0.25 on cos
rows -- per-partition scale/bias columns), range-reduced with the fp32
magic-constant round trick r = u - ((u + 1.5*2^23) - 1.5*2^23) in
[-0.5, 0.5], then Sin(2*pi*r). The tensor_tensor subtract runs on GpSimd
(otherwise idle) when GPS_TT is set, relieving the DVE.

Matmuls run in float32r (FP22) -- full bf16-rate at moving-dim >= 256,
mantissa 13 bits; end-to-end sim incl. f22 truncation: rel err 6.9e-3
vs the 2e-2 gate.

Sharding: rows of x across 8 cores (256 each), X2 replicated. Host-side
prep is layout-only (transpose + 8x partition replication) plus the tiny
12-coefficient fit (data-independent sizes).
"""

from contextlib import ExitStack

import numpy as np

import concourse.bass as bass
import concourse.tile as tile
from concourse import mybir, bacc
from concourse.bass_utils import run_bass_kernel_spmd

AF = mybir.ActivationFunctionType
OP = mybir.AluOpType
F32 = mybir.dt.float32
F32R = mybir.dt.float32r

N, M, D = 2048, 2048, 16
NCORES = 8
NS = N // NCORES          # 256 rows of x per core
P = 128                   # SBUF partitions
NIT = NS // P             # 2 i-blocks per core
JW = 512                  # PSUM bank width (fp32)
NJC = M // JW             # 4 j-chunks
KF = 12                   # cosine frequencies
CT = (2 * KF * D) // P    # 3 contraction tiles of 128
L_FAC = 1.2               # fit period factor: w_k = pi k / (L_FAC * A)
MAGIC = 12582912.0        # 1.5 * 2^23: fp32 round-to-nearest-int trick
TWO_PI = float(2.0 * np.pi)
MINUS_PI = float(-np.pi)
ZOFF = 8.0                # keeps u = w*v + ph'' nonnegative before mod

# layout of per-core constants: colsf [128, CFW] f32 (engine scalars),
# colsr [128, CRW] f32r (matmul lhsT vectors, host-truncated to fp22)
CW = 0        # 0..2   w'_k (turns per unit) per c-tile
CPH = 3       # 3..5   phase (0.25 cos rows / 0 sin rows)
CA = 6        # 6..8   -(a_k/2)  (phi feature scale)
CA0 = 9       # 8*a0 replicated
CMPI = 10     # -pi replicated (Sin bias AP)
CFW = 11
CV1 = 0       # 0..2   t1 lhsT vector (-1 on cos rows)
CV2 = 3       # 3..5   t2 lhsT vector (a_k/2 on cos rows)
CRW = 6

_CACHE = {}
# ablation switches for perf experiments: subsets of
# {"feat", "mms", "extras", "epilogue"} are DISABLED when present
ABLATE = set()
# run the range-reduction tensor_tensor subtract on GpSimd instead of DVE
# (measured ~16x slower there -- gpsimd serializes its 16 partitions/core)
GPS_TT = False


from contextlib import contextmanager


@contextmanager
def _pinned_act_tables():
    """Pin Sin/Copy to the single `trig_and_small` set so the act-table-load
    pass emits exactly one load. Scoped: restored right after compile."""
    import concourse.hw_specs as hw_specs
    import concourse.bacc as bacc_mod

    orig = bacc_mod.get_activation_tables
    base = hw_specs.get_activation_tables
    ours = {AF.Sin, AF.Identity, AF.Copy, AF.Abs, AF.Square}

    def patched(module_arch):
        tabs = {k: set(v) for k, v in base(module_arch).items()}
        for name, fns in tabs.items():
            if name != "trig_and_small":
                fns -= ours
        return tabs

    bacc_mod.get_activation_tables = patched
    try:
        yield
    finally:
        bacc_mod.get_activation_tables = orig


def _build_nc(reps=1, body_reps=1):
    with _pinned_act_tables():
        return _build_nc_inner(reps, body_reps)


def _build_nc_inner(reps=1, body_reps=1):
    nc = bacc.Bacc(trn_type="TRN2", target_bir_lowering=False, debug=False,
                   num_devices=NCORES)

    yrep = nc.declare_dram_parameter("yrep", [P, M], F32, isOutput=False)
    xrep = nc.declare_dram_parameter("xrep", [P, NS], F32, isOutput=False)
    colsf = nc.declare_dram_parameter("colsf", [P, CFW], F32, isOutput=False)
    colsr = nc.declare_dram_parameter("colsr", [P, CRW], F32R, isOutput=False)
    ones = nc.declare_dram_parameter("ones", [2, M], F32R, isOutput=False)
    out = nc.declare_dram_parameter("out", [NS, M], F32, isOutput=True)

    aps = tuple(h.ap() for h in (yrep, xrep, colsf, colsr, ones, out))

    with tile.TileContext(nc) as tc, ExitStack() as ctx:
        pools = {
            "const": ctx.enter_context(tc.tile_pool(name="const", bufs=2)),
            "xs": ctx.enter_context(tc.tile_pool(name="xs", bufs=2)),
            "ys": ctx.enter_context(tc.tile_pool(name="ys", bufs=2)),
            "phi": ctx.enter_context(tc.tile_pool(name="phi", bufs=2)),
            "psi": ctx.enter_context(tc.tile_pool(name="psi", bufs=2)),
            "stage": ctx.enter_context(tc.tile_pool(name="stage", bufs=2)),
            "ps": ctx.enter_context(tc.tile_pool(name="ps", bufs=1,
                                                 space="PSUM")),
        }

        if reps > 1:  # benchmark mode: repeat the whole body on-device
            ctx.enter_context(tc.For_i(0, reps, 1, staggered_reset=True))

        for _ in range(body_reps):
            _emit_body(nc, tc, pools, aps)

    nc.compile()
    return nc


def _emit_body(nc, tc, pools, aps):
    yrep_ap, xrep_ap, colsf_ap, colsr_ap, ones2_ap, out_ap = aps
    ones_ap = ones2_ap[0:1, :]
    a0row_ap = ones2_ap[1:2, :]
    const = pools["const"]

    # ---- inputs (yrep first: biggest and heads the sync-ring FIFO) ----
    yrepb = const.tile([P, M], F32, tag="yrep")
    nc.sync.dma_start(out=yrepb, in_=yrep_ap)
    colsb = const.tile([P, CFW], F32, tag="colsf")
    nc.sync.dma_start(out=colsb, in_=colsf_ap)
    colsr = const.tile([P, CRW], F32R, tag="colsr")
    nc.sync.dma_start(out=colsr, in_=colsr_ap)
    xrepb = const.tile([P, NS], F32, tag="xrep")
    nc.sync.dma_start(out=xrepb, in_=xrep_ap)

    # extras rows (engine partition bases must be 32-aligned, so rows >= 1
    # are filled by DMA): lxe = [ones, t1row, ones], rye = [raw t2 row
    # (= 0.5 t2 - 8 a0), ones, 8*a0] -- row2 pairs restore the a0 constant
    lxe = const.tile([3, NS], F32R, tag="lxe")
    rye = const.tile([3, M], F32R, tag="rye")
    nc.sync.dma_start(out=lxe[0:1, :], in_=ones_ap[0:1, 0:NS])
    nc.sync.dma_start(out=lxe[2:3, :], in_=ones_ap[0:1, 0:NS])
    nc.sync.dma_start(out=rye[1:2, :], in_=ones_ap)
    nc.sync.dma_start(out=rye[2:3, :], in_=a0row_ap)

    # ---- features: per c-tile t, partition p encodes (k, trig, d) ----
    phis, psis = [], []
    for t in range(CT):
        if "feat" in ABLATE:
            phi = pools["phi"].tile([P, NS], F32R, tag=f"phi{t}")
            nc.gpsimd.memset(phi, 0.25)
            phis.append(phi)
            psi = pools["psi"].tile([P, M], F32R, tag=f"psi{t}")
            nc.gpsimd.memset(psi, 0.25)
            psis.append(psi)
            continue
        wc = colsb[:, CW + t:CW + t + 1]
        phc = colsb[:, CPH + t:CPH + t + 1]
        ac = colsb[:, CA + t:CA + t + 1]

        # x side (small): same chain + coefficient scale, all on DVE
        ux = pools["xs"].tile([P, NS], F32, tag="ux")
        rx = pools["xs"].tile([P, NS], F32, tag="rx")
        nc.vector.tensor_scalar(out=ux, in0=xrepb, scalar1=wc, scalar2=phc,
                                op0=OP.mult, op1=OP.add)
        nc.vector.tensor_scalar(out=rx, in0=ux, scalar1=MAGIC, scalar2=MAGIC,
                                op0=OP.add, op1=OP.subtract)
        nc.vector.tensor_tensor(out=ux, in0=ux, in1=rx, op=OP.subtract)
        phi = pools["phi"].tile([P, NS], F32R, tag=f"phi{t}")
        nc.scalar.activation(out=phi, in_=ux, func=AF.Sin, scale=TWO_PI)
        nc.vector.tensor_scalar(out=phi, in0=phi, scalar1=ac, scalar2=None,
                                op0=OP.mult)
        phis.append(phi)

        # y side: u = w*y + ph (DVE 2-op), rnd = (u+M)-M (DVE 2-op),
        # r = u - rnd (GpSimd or DVE), Sin on ACT
        uy = pools["ys"].tile([P, M], F32, tag="uy")
        ry = pools["ys"].tile([P, M], F32, tag="ry")
        nc.scalar.activation(out=uy, in_=yrepb, func=AF.Identity,
                             bias=phc, scale=wc)
        nc.vector.tensor_scalar(out=ry, in0=uy, scalar1=MAGIC, scalar2=MAGIC,
                                op0=OP.add, op1=OP.subtract)
        tt_eng = nc.gpsimd if GPS_TT else nc.vector
        tt_eng.tensor_tensor(out=uy, in0=uy, in1=ry, op=OP.subtract)
        psi = pools["psi"].tile([P, M], F32R, tag=f"psi{t}")
        nc.scalar.activation(out=psi, in_=uy, func=AF.Sin, scale=TWO_PI)
        psis.append(psi)

    if "extras" in ABLATE:
        nc.gpsimd.memset(lxe, 0.5)
        nc.gpsimd.memset(rye, 0.5)
        _skip_extras = True
    else:
        _skip_extras = False
    # ---- t1 row: p1[0, i] = 0.5*t1_i - 8*a0; staged at partition 0,
    # then a tiny SBUF->SBUF DMA drops it into lxe partition 1 ----
    if not _skip_extras:
        p1 = pools["ps"].tile([1, NS], F32, tag="p1")
        for t in range(CT):
            nc.tensor.matmul(p1, colsr[:, CV1 + t:CV1 + t + 1], phis[t],
                             start=(t == 0), stop=(t == CT - 1))
        t1row = const.tile([1, NS], F32R, tag="t1row")
        nc.vector.tensor_copy(t1row, p1)
        nc.scalar.dma_start(out=lxe[1:2, :], in_=t1row)

    # ---- t2 row: rye[0, j] = 0.5*t2_j, two rotating [1, JW] banks ----
    if not _skip_extras:
        for jc in range(NJC):
            p2 = pools["ps"].tile([1, JW], F32, name=f"p2{jc}",
                                  tag=f"p2{jc % 2}")
            for t in range(CT):
                nc.tensor.matmul(p2, colsr[:, CV2 + t:CV2 + t + 1],
                                 psis[t][:, jc * JW:(jc + 1) * JW],
                                 start=(t == 0), stop=(t == CT - 1))
            nc.vector.tensor_copy(rye[0:1, jc * JW:(jc + 1) * JW], p2)

    # ---- main matmuls: bank (it, jc) accumulates 3 trig tiles + extras.
    # 6 rotating PSUM bank tags (+p1 +p2 = 8 total); epilogue per i-block
    # so reused banks are drained before the second block's matmuls. ----
    for it in range(NIT):
        banks = {}
        for c in range(CT + 1):
            if "mms" in ABLATE and c > 0:
                continue
            for jc in range(NJC):
                if jc not in banks:
                    n = it * NJC + jc
                    banks[jc] = pools["ps"].tile([P, JW], F32,
                                                 name=f"b{n % 5}",
                                                 tag=f"b{n % 5}")
                b = banks[jc]
                if "mms" in ABLATE:
                    nc.tensor.matmul(
                        b, lxe[:, it * P:(it + 1) * P],
                        rye[:, jc * JW:(jc + 1) * JW],
                        start=True, stop=True)
                elif c < CT:
                    nc.tensor.matmul(
                        b, phis[c][:, it * P:(it + 1) * P],
                        psis[c][:, jc * JW:(jc + 1) * JW],
                        start=(c == 0),
                        stop=(_skip_extras and c == CT - 1))
                elif not _skip_extras:
                    nc.tensor.matmul(
                        b, lxe[:, it * P:(it + 1) * P],
                        rye[:, jc * JW:(jc + 1) * JW],
                        start=False, stop=True)

        # epilogue: PSUM -> SBUF -> DRAM, split copies DVE/ACT
        for jc in range(NJC):
            st = pools["stage"].tile([P, JW], F32, name=f"st{it}_{jc}",
                                     tag=f"st{it}_{jc}")
            if jc % 2 == 0 or (it == 1 and jc == 1):
                nc.scalar.copy(st, banks[jc])
            else:
                nc.vector.tensor_copy(st, banks[jc])
            nc.scalar.dma_start(
                out=out_ap[it * P:(it + 1) * P, jc * JW:(jc + 1) * JW],
                in_=st)


def _get_nc(reps=1, body_reps=1):
    key = ("nc", reps, body_reps, tuple(sorted(ABLATE)), GPS_TT)
    if key not in _CACHE:
        _CACHE[key] = _build_nc(reps, body_reps)
    return _CACHE[key]


def _fit(p, A):
    """Minimax-ish (Lawson IRLS) cosine fit of t^p on [0, A]."""
    key = ("fit", round(float(p), 8), round(float(A), 5))
    if key in _CACHE:
        return _CACHE[key]
    t = np.linspace(0.0, A, 4001)
    f = t ** p
    w = np.pi * np.arange(1, KF + 1) / (L_FAC * A)
    Mx = np.concatenate([np.ones((len(t), 1)), np.cos(np.outer(t, w))], axis=1)
    wt = np.ones(len(t))
    coef = None
    for _ in range(60):
        coef, *_ = np.linalg.lstsq(Mx * wt[:, None], f * wt, rcond=None)
        r = Mx @ coef - f
        wt *= np.abs(r) + 1e-12
        wt /= wt.sum()
        wt = np.maximum(wt, 1e-14)
    _CACHE[key] = (float(coef[0]), coef[1:].copy(), w)
    return _CACHE[key]


def _make_in_maps(x, X2, log_H):
    x = np.ascontiguousarray(np.asarray(x, dtype=np.float32))
    X2 = np.ascontiguousarray(np.asarray(X2, dtype=np.float32))
    lh = float(np.asarray(log_H, dtype=np.float32))
    p = 2.0 * float(np.log1p(np.exp(lh)))
    A = float(np.abs(x).max() + np.abs(X2).max())
    a0, a, w = _fit(p, A)
    wt = (w / (2.0 * np.pi)).astype(np.float64)   # turns per unit

    cc = np.arange(2 * KF * D)
    k_of = cc // 32
    trig = (cc % 32) // 16                        # 0 = cos, 1 = sin
    colsf = np.zeros((P, CFW), np.float32)
    colsr = np.zeros((P, CRW), np.float32)
    for t in range(CT):
        s = slice(t * P, (t + 1) * P)
        colsf[:, CW + t] = wt[k_of[s]]
        colsf[:, CPH + t] = np.where(trig[s] == 0, 0.25, 0.0)
        colsf[:, CA + t] = -(a[k_of[s]] / 2.0)
        colsr[:, CV1 + t] = np.where(trig[s] == 0, -1.0, 0.0)
        colsr[:, CV2 + t] = np.where(trig[s] == 0, a[k_of[s]] / 2.0, 0.0)
    colsf[:, CA0] = 8.0 * a0
    colsf[:, CMPI] = MINUS_PI
    # truncate matmul weight vectors to fp22 so the bits are valid f32r
    colsr.view(np.uint32)[...] &= np.uint32(0xFFFFFC00)

    yrep = np.ascontiguousarray(np.tile(X2.T, (P // D, 1)))   # [128, 2048]
    in_maps = []
    for c in range(NCORES):
        xs = x[c * NS:(c + 1) * NS]
        xrep = np.ascontiguousarray(np.tile(xs.T, (P // D, 1)))  # [128, 256]
        ones2 = np.ones((2, M), np.float32)
        ones2[1, :] = 8.0 * a0
        ones2.view(np.uint32)[...] &= np.uint32(0xFFFFFC00)
        in_maps.append({"yrep": yrep, "xrep": xrep, "colsf": colsf,
                        "colsr": colsr, "ones": ones2})
    return in_maps


def run_spmd(x, X2, log_H, trace=False, reps=1, body_reps=1, **kw):
    nc = _get_nc(reps, body_reps)
    in_maps = _make_in_maps(x, X2, log_H)
    return run_bass_kernel_spmd(nc, in_maps, list(range(NCORES)),
                                trace=trace, **kw)


def kernel(x, X2, log_H):
    res = run_spmd(x, X2, log_H)
    return np.concatenate([res.results[c]["out"] for c in range(NCORES)],
                          axis=0)



# revision 28
# speedup vs baseline: 1.4179x; 1.2210x over previous
"""Trainium2 Bass kernel: fractional Brownian motion kernel layer.

K[i,j] = 0.5 * sum_d (|x_id|^p + |X2_jd|^p - |x_id - X2_jd|^p),
p = 2*softplus(log_H),  x:[2048,16], X2:[2048,16] -> K:[2048,2048] f32.

Strategy: cosine-feature factorization onto the tensor engine. The 1-D
function f(t) = |t|^p is approximated on [-A, A] (A = max|x| + max|X2|,
fit solved on host per call) by

    f(t) ~= a0 + sum_k a_k cos(w_k t),   k = 1..12,  w_k = pi k / (1.2 A)

and cos(w(x-y)) = cos(wx)cos(wy) + sin(wx)sin(wy) is exactly separable, so

    K[i,j] = [0.5 t1_i - 8 a0]*1_j + 1_i*[0.5 t2_j] - sum_{d,k} (a_k/2)
             (cos(w_k x_id)cos(w_k X2_jd) + sin(w_k x_id)sin(w_k X2_jd))

which is ONE matmul with contraction 2*12*16 = 384 (3 tiles of 128) plus a
rank-2 "extras" matmul carrying the t1/t2 rows -- themselves produced by
tiny PE matmuls against the same feature tiles. PSUM accumulates all four
contraction passes; each [128,512] bank IS the final output tile.

Trig features: the ACT Sin spline is only valid on [-pi, pi] (HW-probed), so
arguments are computed in turn units u = w'_k v + phase (phase 0.25 on cos
rows -- per-partition scale/bias columns), range-reduced with the fp32
magic-constant round trick r = u - ((u + 1.5*2^23) - 1.5*2^23) in
[-0.5, 0.5], then Sin(2*pi*r). The tensor_tensor subtract runs on GpSimd
(otherwise idle) when GPS_TT is set, relieving the DVE.

Matmuls run in float32r (FP22) -- full bf16-rate at moving-dim >= 256,
mantissa 13 bits; end-to-end sim incl. f22 truncation: rel err 6.9e-3
vs the 2e-2 gate.

Sharding: rows of x across 8 cores (256 each), X2 replicated. Host-side
prep is layout-only (transpose + 8x partition replication) plus the tiny
12-coefficient fit (data-independent sizes).
"""

from contextlib import ExitStack

import numpy as np

import concourse.bass as bass
import concourse.tile as tile
from concourse import mybir, bacc
from concourse.bass_utils import run_bass_kernel_spmd

AF = mybir.ActivationFunctionType
OP = mybir.AluOpType
F32 = mybir.dt.float32
F32R = mybir.dt.float32r

N, M, D = 2048, 2048, 16
NCORES = 8
NS = N // NCORES          # 256 rows of x per core
P = 128                   # SBUF partitions
NIT = NS // P             # 2 i-blocks per core
JW = 512                  # PSUM bank width (fp32)
NJC = M // JW             # 4 j-chunks
KF = 12                   # cosine frequencies
CT = (2 * KF * D) // P    # 3 contraction tiles of 128
L_FAC = 1.2               # fit period factor: w_k = pi k / (L_FAC * A)
MAGIC = 12582912.0        # 1.5 * 2^23: fp32 round-to-nearest-int trick
TWO_PI = float(2.0 * np.pi)
MINUS_PI = float(-np.pi)
ZOFF = 8.0                # keeps u = w*v + ph'' nonnegative before mod

# layout of per-core constants: colsf [128, CFW] f32 (engine scalars),
# colsr [128, CRW] f32r (matmul lhsT vectors, host-truncated to fp22)
CW = 0        # 0..2   w'_k (turns per unit) per c-tile
CPH = 3       # 3..5   phase (0.25 cos rows / 0 sin rows)
CA = 6        # 6..8   -(a_k/2)  (phi feature scale)
CA0 = 9       # 8*a0 replicated
CMPI = 10     # -pi replicated (Sin bias AP)
CFW = 11
CV1 = 0       # 0..2   t1 lhsT vector (-1 on cos rows)
CV2 = 3       # 3..5   t2 lhsT vector (a_k/2 on cos rows)
CRW = 6

_CACHE = {}
# ablation switches for perf experiments: subsets of
# {"feat", "mms", "extras", "epilogue"} are DISABLED when present
ABLATE = set()
# run the range-reduction tensor_tensor subtract on GpSimd instead of DVE
# (measured ~16x slower there -- gpsimd serializes its 16 partitions/core)
GPS_TT = False


from contextlib import contextmanager


@contextmanager
def _pinned_act_tables():
    """Pin Sin/Copy to the single `trig_and_small` set so the act-table-load
    pass emits exactly one load. Scoped: restored right after compile."""
    import concourse.hw_specs as hw_specs
    import concourse.bacc as bacc_mod

    orig = bacc_mod.get_activation_tables
    base = hw_specs.get_activation_tables
    ours = {AF.Sin, AF.Identity, AF.Copy, AF.Abs, AF.Square}

    def patched(module_arch):
        tabs = {k: set(v) for k, v in base(module_arch).items()}
        for name, fns in tabs.items():
            if name != "trig_and_small":
                fns -= ours
        return tabs

    bacc_mod.get_activation_tables = patched
    try:
        yield
    finally:
        bacc_mod.get_activation_tables = orig


def _build_nc(reps=1, body_reps=1):
    with _pinned_act_tables():
        return _build_nc_inner(reps, body_reps)


def _build_nc_inner(reps=1, body_reps=1):
    nc = bacc.Bacc(trn_type="TRN2", target_bir_lowering=False, debug=False,
                   num_devices=NCORES)

    yrep = nc.declare_dram_parameter("yrep", [P, M], F32, isOutput=False)
    xrep = nc.declare_dram_parameter("xrep", [P, NS], F32, isOutput=False)
    colsf = nc.declare_dram_parameter("colsf", [P, CFW], F32, isOutput=False)
    colsr = nc.declare_dram_parameter("colsr", [P, CRW], F32R, isOutput=False)
    ones = nc.declare_dram_parameter("ones", [2, M], F32R, isOutput=False)
    out = nc.declare_dram_parameter("out", [NS, M], F32, isOutput=True)

    aps = tuple(h.ap() for h in (yrep, xrep, colsf, colsr, ones, out))

    with tile.TileContext(nc) as tc, ExitStack() as ctx:
        pools = {
            "const": ctx.enter_context(tc.tile_pool(name="const", bufs=2)),
            "xs": ctx.enter_context(tc.tile_pool(name="xs", bufs=2)),
            "ys": ctx.enter_context(tc.tile_pool(name="ys", bufs=2)),
            "phi": ctx.enter_context(tc.tile_pool(name="phi", bufs=2)),
            "psi": ctx.enter_context(tc.tile_pool(name="psi", bufs=2)),
            "stage": ctx.enter_context(tc.tile_pool(name="stage", bufs=2)),
            "ps": ctx.enter_context(tc.tile_pool(name="ps", bufs=1,
                                                 space="PSUM")),
        }

        if reps > 1:  # benchmark mode: repeat the whole body on-device
            ctx.enter_context(tc.For_i(0, reps, 1, staggered_reset=True))

        for _ in range(body_reps):
            _emit_body(nc, tc, pools, aps)

    nc.compile()
    return nc


def _emit_body(nc, tc, pools, aps):
    yrep_ap, xrep_ap, colsf_ap, colsr_ap, ones2_ap, out_ap = aps
    ones_ap = ones2_ap[0:1, :]
    a0row_ap = ones2_ap[1:2, :]
    const = pools["const"]

    # ---- inputs (yrep first: biggest and heads the sync-ring FIFO) ----
    yrepb = const.tile([P, M], F32, tag="yrep")
    nc.sync.dma_start(out=yrepb, in_=yrep_ap)
    colsb = const.tile([P, CFW], F32, tag="colsf")
    nc.sync.dma_start(out=colsb, in_=colsf_ap)
    colsr = const.tile([P, CRW], F32R, tag="colsr")
    nc.sync.dma_start(out=colsr, in_=colsr_ap)
    xrepb = const.tile([P, NS], F32, tag="xrep")
    nc.sync.dma_start(out=xrepb, in_=xrep_ap)

    # extras rows (engine partition bases must be 32-aligned, so rows >= 1
    # are filled by DMA): lxe = [ones, t1row, ones], rye = [raw t2 row
    # (= 0.5 t2 - 8 a0), ones, 8*a0] -- row2 pairs restore the a0 constant
    lxe = const.tile([3, NS], F32R, tag="lxe")
    rye = const.tile([3, M], F32R, tag="rye")
    nc.sync.dma_start(out=lxe[0:1, :], in_=ones_ap[0:1, 0:NS])
    nc.sync.dma_start(out=lxe[2:3, :], in_=ones_ap[0:1, 0:NS])
    nc.sync.dma_start(out=rye[1:2, :], in_=ones_ap)
    nc.sync.dma_start(out=rye[2:3, :], in_=a0row_ap)

    # ---- features: per c-tile t, partition p encodes (k, trig, d) ----
    phis, psis = [], []
    for t in range(CT):
        if "feat" in ABLATE:
            phi = pools["phi"].tile([P, NS], F32R, tag=f"phi{t}")
            nc.gpsimd.memset(phi, 0.25)
            phis.append(phi)
            psi = pools["psi"].tile([P, M], F32R, tag=f"psi{t}")
            nc.gpsimd.memset(psi, 0.25)
            psis.append(psi)
            continue
        wc = colsb[:, CW + t:CW + t + 1]
        phc = colsb[:, CPH + t:CPH + t + 1]
        ac = colsb[:, CA + t:CA + t + 1]

        # x side (small): same chain + coefficient scale, all on DVE
        ux = pools["xs"].tile([P, NS], F32, tag="ux")
        rx = pools["xs"].tile([P, NS], F32, tag="rx")
        nc.vector.tensor_scalar(out=ux, in0=xrepb, scalar1=wc, scalar2=phc,
                                op0=OP.mult, op1=OP.add)
        nc.vector.tensor_scalar(out=rx, in0=ux, scalar1=MAGIC, scalar2=MAGIC,
                                op0=OP.add, op1=OP.subtract)
        nc.vector.tensor_tensor(out=ux, in0=ux, in1=rx, op=OP.subtract)
        phi = pools["phi"].tile([P, NS], F32R, tag=f"phi{t}")
        nc.scalar.activation(out=phi, in_=ux, func=AF.Sin, scale=TWO_PI)
        nc.vector.tensor_scalar(out=phi, in0=phi, scalar1=ac, scalar2=None,
                                op0=OP.mult)
        phis.append(phi)

        # y side: u = w*y + ph (DVE 2-op), rnd = (u+M)-M (DVE 2-op),
        # r = u - rnd (GpSimd or DVE), Sin on ACT
        uy = pools["ys"].tile([P, M], F32, tag="uy")
        ry = pools["ys"].tile([P, M], F32, tag="ry")
        psi = pools["psi"].tile([P, M], F32R, tag=f"psi{t}")
        tt_eng = nc.gpsimd if GPS_TT else nc.vector
        # j-halves so ACT (Identity/Sin) and DVE (round/subtract) overlap
        # within a tile instead of serializing on the full 2048-wide chain
        for h in range(2):
            sl = slice(h * (M // 2), (h + 1) * (M // 2))
            nc.scalar.activation(out=uy[:, sl], in_=yrepb[:, sl],
                                 func=AF.Identity, bias=phc, scale=wc)
            nc.vector.tensor_scalar(out=ry[:, sl], in0=uy[:, sl],
                                    scalar1=MAGIC, scalar2=MAGIC,
                                    op0=OP.add, op1=OP.subtract)
            tt_eng.tensor_tensor(out=uy[:, sl], in0=uy[:, sl],
                                 in1=ry[:, sl], op=OP.subtract)
            nc.scalar.activation(out=psi[:, sl], in_=uy[:, sl], func=AF.Sin,
                                 scale=TWO_PI)
        psis.append(psi)

    if "extras" in ABLATE:
        nc.gpsimd.memset(lxe, 0.5)
        nc.gpsimd.memset(rye, 0.5)
        _skip_extras = True
    else:
        _skip_extras = False
    # ---- t1 row: p1[0, i] = 0.5*t1_i - 8*a0; staged at partition 0,
    # then a tiny SBUF->SBUF DMA drops it into lxe partition 1 ----
    if not _skip_extras:
        p1 = pools["ps"].tile([1, NS], F32, tag="p1")
        for t in range(CT):
            nc.tensor.matmul(p1, colsr[:, CV1 + t:CV1 + t + 1], phis[t],
                             start=(t == 0), stop=(t == CT - 1))
        t1row = const.tile([1, NS], F32R, tag="t1row")
        nc.vector.tensor_copy(t1row, p1)
        nc.scalar.dma_start(out=lxe[1:2, :], in_=t1row)

    # ---- t2 row: rye[0, j] = 0.5*t2_j, two rotating [1, JW] banks ----
    if not _skip_extras:
        for jc in range(NJC):
            p2 = pools["ps"].tile([1, JW], F32, name=f"p2{jc}",
                                  tag=f"p2{jc % 2}")
            for t in range(CT):
                nc.tensor.matmul(p2, colsr[:, CV2 + t:CV2 + t + 1],
                                 psis[t][:, jc * JW:(jc + 1) * JW],
                                 start=(t == 0), stop=(t == CT - 1))
            nc.vector.tensor_copy(rye[0:1, jc * JW:(jc + 1) * JW], p2)

    # ---- main matmuls: bank (it, jc) accumulates 3 trig tiles + extras.
    # 6 rotating PSUM bank tags (+p1 +p2 = 8 total); epilogue per i-block
    # so reused banks are drained before the second block's matmuls. ----
    for it in range(NIT):
        banks = {}
        for c in range(CT + 1):
            if "mms" in ABLATE and c > 0:
                continue
            for jc in range(NJC):
                if jc not in banks:
                    n = it * NJC + jc
                    banks[jc] = pools["ps"].tile([P, JW], F32,
                                                 name=f"b{n % 5}",
                                                 tag=f"b{n % 5}")
                b = banks[jc]
                if "mms" in ABLATE:
                    nc.tensor.matmul(
                        b, lxe[:, it * P:(it + 1) * P],
                        rye[:, jc * JW:(jc + 1) * JW],
                        start=True, stop=True)
                elif c < CT:
                    nc.tensor.matmul(
                        b, phis[c][:, it * P:(it + 1) * P],
                        psis[c][:, jc * JW:(jc + 1) * JW],
                        start=(c == 0),
                        stop=(_skip_extras and c == CT - 1))
                elif not _skip_extras:
                    nc.tensor.matmul(
                        b, lxe[:, it * P:(it + 1) * P],
                        rye[:, jc * JW:(jc + 1) * JW],
                        start=False, stop=True)

        # epilogue: PSUM -> SBUF -> DRAM, split copies DVE/ACT
        for jc in range(NJC):
            st = pools["stage"].tile([P, JW], F32, name=f"st{it}_{jc}",
                                     tag=f"st{it}_{jc}")
            if jc % 2 == 0 or (it == 1 and jc == 1):
                nc.scalar.copy(st, banks[jc])
            else:
                nc.vector.tensor_copy(st, banks[jc])
            nc.scalar.dma_start(
                out=out_ap[it * P:(it + 1) * P, jc * JW:(jc + 1) * JW],
                in_=st)


def _get_nc(reps=1, body_reps=1):
    key = ("nc", reps, body_reps, tuple(sorted(ABLATE)), GPS_TT)
    if key not in _CACHE:
        _CACHE[key] = _build_nc(reps, body_reps)
    return _CACHE[key]


def _fit(p, A):
    """Minimax-ish (Lawson IRLS) cosine fit of t^p on [0, A]."""
    key = ("fit", round(float(p), 8), round(float(A), 5))
    if key in _CACHE:
        return _CACHE[key]
    t = np.linspace(0.0, A, 4001)
    f = t ** p
    w = np.pi * np.arange(1, KF + 1) / (L_FAC * A)
    Mx = np.concatenate([np.ones((len(t), 1)), np.cos(np.outer(t, w))], axis=1)
    wt = np.ones(len(t))
    coef = None
    for _ in range(60):
        coef, *_ = np.linalg.lstsq(Mx * wt[:, None], f * wt, rcond=None)
        r = Mx @ coef - f
        wt *= np.abs(r) + 1e-12
        wt /= wt.sum()
        wt = np.maximum(wt, 1e-14)
    _CACHE[key] = (float(coef[0]), coef[1:].copy(), w)
    return _CACHE[key]


def _make_in_maps(x, X2, log_H):
    x = np.ascontiguousarray(np.asarray(x, dtype=np.float32))
    X2 = np.ascontiguousarray(np.asarray(X2, dtype=np.float32))
    lh = float(np.asarray(log_H, dtype=np.float32))
    p = 2.0 * float(np.log1p(np.exp(lh)))
    A = float(np.abs(x).max() + np.abs(X2).max())
    a0, a, w = _fit(p, A)
    wt = (w / (2.0 * np.pi)).astype(np.float64)   # turns per unit

    cc = np.arange(2 * KF * D)
    k_of = cc // 32
    trig = (cc % 32) // 16                        # 0 = cos, 1 = sin
    colsf = np.zeros((P, CFW), np.float32)
    colsr = np.zeros((P, CRW), np.float32)
    for t in range(CT):
        s = slice(t * P, (t + 1) * P)
        colsf[:, CW + t] = wt[k_of[s]]
        colsf[:, CPH + t] = np.where(trig[s] == 0, 0.25, 0.0)
        colsf[:, CA + t] = -(a[k_of[s]] / 2.0)
        colsr[:, CV1 + t] = np.where(trig[s] == 0, -1.0, 0.0)
        colsr[:, CV2 + t] = np.where(trig[s] == 0, a[k_of[s]] / 2.0, 0.0)
    colsf[:, CA0] = 8.0 * a0
    colsf[:, CMPI] = MINUS_PI
    # truncate matmul weight vectors to fp22 so the bits are valid f32r
    colsr.view(np.uint32)[...] &= np.uint32(0xFFFFFC00)

    yrep = np.ascontiguousarray(np.tile(X2.T, (P // D, 1)))   # [128, 2048]
    in_maps = []
    for c in range(NCORES):
        xs = x[c * NS:(c + 1) * NS]
        xrep = np.ascontiguousarray(np.tile(xs.T, (P // D, 1)))  # [128, 256]
        ones2 = np.ones((2, M), np.float32)
        ones2[1, :] = 8.0 * a0
        ones2.view(np.uint32)[...] &= np.uint32(0xFFFFFC00)
        in_maps.append({"yrep": yrep, "xrep": xrep, "colsf": colsf,
                        "colsr": colsr, "ones": ones2})
    return in_maps


def run_spmd(x, X2, log_H, trace=False, reps=1, body_reps=1, **kw):
    nc = _get_nc(reps, body_reps)
    in_maps = _make_in_maps(x, X2, log_H)
    return run_bass_kernel_spmd(nc, in_maps, list(range(NCORES)),
                                trace=trace, **kw)


def kernel(x, X2, log_H):
    res = run_spmd(x, X2, log_H)
    return np.concatenate([res.results[c]["out"] for c in range(NCORES)],
                          axis=0)

